# revision 18
# baseline (speedup 1.0000x reference)
"""Trainium2 Bass kernel for the DVS-SNN problem.

Model (per reference):
    for t in 0..T-1:
        i1 = x_t @ w1.T + spk @ w_rec.T
        v1 = v1 + i1 - LEAK ; spk = (v1 >= 1) ; v1 -= spk
        v2 = max(v2 + spk @ w2.T - OUTPUT_LEAK, 0) ; out_sum += v2
    return out_sum / T

Strategy: data-parallel over batch (64 = 8 cores x 8). Per core:
  Phase A (time-parallel): XprojT[h, t*8+b] = scale*((x_t @ w1.T).T - LEAK)
      with x pre-transposed on host to xt[c, t*8+b]; fp16 hi/lo 3-pass
      (exact to ~fp32). Runs in j-pair passes, double-buffered PSUM, and
      interleaves into phase B's PE gaps (Tile scheduler).
  Phase B (sequential scan over T): state transposed [H-chunkwise on
      partitions, B on free]: tiles [128, 32], free = 8*h_chunk + b.
      rec.T = w_rec @ spk.T via 16 matmuls (stationary = w_rec.T chunks,
      fp16 scaled by 64 to stay out of subnormals; v1 dynamics scaled by
      64, numerically transparent). Burst is ordered k-outer so the next
      step's first matmuls only need the first half of the new spikes.
      Spikes are written straight into spk_store (fp16) which feeds both
      the matmuls and phase C.
  Phase C (time-parallel): i2 = spk @ w2.T batched; the v2 relu
      accumulator is a first-order recurrence = one tensor_tensor_scan
      (state = max(state + d_t, 0)) over [B*O, T] layout; reduce-sum
      gives out_sum.

Modes (env SNN_MODE): "fp32" exact | "fp16" (default, ~1.7e-3).
"""

import os
import numpy as np

B, T, C, H, O = 64, 256, 2048, 512, 11
THRESHOLD = 1.0
LEAK = 0.003
OUTPUT_LEAK = LEAK * 0.5

NCORES = 8
BL = B // NCORES          # batch per core = 8
BT = T * BL               # 2048 moving columns per core
P = 128
KC = C // P               # 16 contraction chunks for phase A
KH = H // P               # 4 H chunks
NTILE = 512               # phase A moving tile (64 timesteps x 8 batch)

MODE = os.environ.get("SNN_MODE", "fp16")

# fp16 mode: scale v1 dynamics so fp16(SCALE*w_rec) avoids subnormals.
SCALE = 1.0 if MODE == "fp32" else 64.0
LO_SHIFT = 2048.0  # 2^11: scaling for the w1 low-part in fp16 3-pass


def build_nc(t_steps=T, mode=MODE, debug_out=False):
    """Build the Bass program (same program for all 8 cores)."""
    from contextlib import ExitStack

    import concourse.tile as tile
    from concourse import bacc, mybir

    f32 = mybir.dt.float32
    f16 = mybir.dt.float16
    wdt = f32 if mode == "fp32" else f16
    alu = mybir.AluOpType
    ACT = mybir.ActivationFunctionType

    nc = bacc.Bacc("TRN2", target_bir_lowering=False, debug=False,
                   num_devices=NCORES)

    # ---- DRAM I/O ----
    if mode == "fp16":
        xh_d = nc.dram_tensor("xh", [C, BT], f16, kind="ExternalInput")
        xl_d = nc.dram_tensor("xl", [C, BT], f16, kind="ExternalInput")
        xs_d = nc.dram_tensor("xs", [C, BT], f16, kind="ExternalInput")
        w1h_d = nc.dram_tensor("w1h", [C, H], f16, kind="ExternalInput")
        w1l_d = nc.dram_tensor("w1l", [C, H], f16, kind="ExternalInput")
    else:
        adt = f32
        xt_d = nc.dram_tensor("xt", [C, BT], adt, kind="ExternalInput")
        w1t_d = nc.dram_tensor("w1t", [C, H], adt, kind="ExternalInput")
    wrt_d = nc.dram_tensor("wrt", [H, H], wdt, kind="ExternalInput")
    w2t_d = nc.dram_tensor("w2t", [H, O], wdt, kind="ExternalInput")
    out_d = nc.dram_tensor("out", [O, BL], f32, kind="ExternalOutput")
    if debug_out:
        v1_d = nc.dram_tensor("v1_dbg", [P, KH * BL], f32, kind="ExternalOutput")
        xp_d = nc.dram_tensor("xp_dbg", [P, 32 * t_steps], f32, kind="ExternalOutput")

    TH_S = SCALE * THRESHOLD
    spk_dt = wdt

    with tile.TileContext(nc) as tc, ExitStack() as ctx:
        # ---- persistent tiles (one pool, one slot per tag) ----
        perm = ctx.enter_context(tc.tile_pool(name="perm", bufs=1))

        def ptile(shape, dt_, tag):
            return perm.tile(shape, dt_, tag=tag, name=tag)

        xproj = ptile([P, 32 * t_steps], f32, "xproj")
        spk_store = ptile([P, 32 * t_steps], spk_dt, "spk_store")
        v1 = ptile([P, KH * BL], f32, "v1")

        w1_sb = []
        if mode == "fp16":
            for k in range(KC):
                w1h_k = ptile([P, H], f16, f"w1h_{k}")
                w1l_k = ptile([P, H], f16, f"w1l_{k}")
                nc.sync.dma_start(out=w1h_k[:], in_=w1h_d.ap()[k * P:(k + 1) * P, :])
                nc.sync.dma_start(out=w1l_k[:], in_=w1l_d.ap()[k * P:(k + 1) * P, :])
                w1_sb.append((w1h_k, w1l_k))
        else:
            for k in range(KC):
                w1_k = ptile([P, H], adt, f"w1_{k}")
                nc.sync.dma_start(out=w1_k[:], in_=w1t_d.ap()[k * P:(k + 1) * P, :])
                w1_sb.append(w1_k)

        wr_sb = []
        w2_sb = []
        for k in range(KH):
            wr_k = ptile([P, H], wdt, f"wr_{k}")
            nc.sync.dma_start(out=wr_k[:], in_=wrt_d.ap()[k * P:(k + 1) * P, :])
            wr_sb.append(wr_k)
            w2_k = ptile([P, O], wdt, f"w2_{k}")
            nc.sync.dma_start(out=w2_k[:], in_=w2t_d.ap()[k * P:(k + 1) * P, :])
            w2_sb.append(w2_k)

        nc.vector.memset(v1[:], 0.0)

        xt_pool = ctx.enter_context(tc.tile_pool(name="xt", bufs=4))
        psA_pool = ctx.enter_context(tc.tile_pool(name="psA", bufs=2, space="PSUM"))
        psB_pool = ctx.enter_context(tc.tile_pool(name="psB", bufs=3, space="PSUM"))

        # ================= Phase A: XprojT =================
        # xproj[p, 32t + 8j + b] = SCALE * ((x_t @ w1.T)[b, 128j+p] - LEAK)
        # j-pair passes: each pass computes 2 of the 4 H-chunks for a
        # column tile (2 PSUM tags x bufs=2 = 4 banks, double-buffered).
        # x tiles are re-streamed once per pass (2x DMA, still hidden).
        # A small leading tile (8 steps) lets phase B start early.
        if t_steps <= NTILE // BL:
            sched = [(0, t_steps)]
        else:
            sched = [(0, 8)]
            s = 8
            while s < t_steps:
                sched.append((s, min(s + NTILE // BL, t_steps)))
                s = min(s + NTILE // BL, t_steps)
        for (s0, s1) in sched:
            ntile = (s1 - s0) * BL
            nsl = slice(s0 * BL, s1 * BL)
            for jp in range(2):
                psA = [psA_pool.tile([P, ntile], f32, tag=f"psA{jj}",
                                     name=f"psA{jj}", padded_shape=[P, NTILE])
                       for jj in range(2)]
                for k in range(KC):
                    csl = slice(k * P, (k + 1) * P)
                    if mode == "fp16":
                        xh_t = xt_pool.tile([P, ntile], f16, tag="xh",
                                            name="xh_t", padded_shape=[P, NTILE])
                        xl_t = xt_pool.tile([P, ntile], f16, tag="xl",
                                            name="xl_t", padded_shape=[P, NTILE])
                        xs_t = xt_pool.tile([P, ntile], f16, tag="xs",
                                            name="xs_t", padded_shape=[P, NTILE])
                        nc.sync.dma_start(out=xh_t[:], in_=xh_d.ap()[csl, nsl])
                        nc.sync.dma_start(out=xl_t[:], in_=xl_d.ap()[csl, nsl])
                        nc.sync.dma_start(out=xs_t[:], in_=xs_d.ap()[csl, nsl])
                        for jj in range(2):
                            j = 2 * jp + jj
                            st_h = w1_sb[k][0][:][:, j * P:(j + 1) * P]
                            st_l = w1_sb[k][1][:][:, j * P:(j + 1) * P]
                            nc.tensor.matmul(psA[jj][:], st_h, xh_t[:],
                                             start=(k == 0), stop=False)
                            nc.tensor.matmul(psA[jj][:], st_h, xl_t[:],
                                             start=False, stop=False)
                            nc.tensor.matmul(psA[jj][:], st_l, xs_t[:],
                                             start=False, stop=(k == KC - 1))
                    else:
                        xt_t = xt_pool.tile([P, ntile], adt, tag="xt",
                                            name="xt_t", padded_shape=[P, NTILE])
                        nc.sync.dma_start(out=xt_t[:], in_=xt_d.ap()[csl, nsl])
                        for jj in range(2):
                            j = 2 * jp + jj
                            st = w1_sb[k][:][:, j * P:(j + 1) * P]
                            nc.tensor.matmul(psA[jj][:], st, xt_t[:],
                                             start=(k == 0), stop=(k == KC - 1))
                # copy PSUM -> xproj (strided dest), fusing scale and leak
                for jj in range(2):
                    j = 2 * jp + jj
                    dest = xproj[:].rearrange("p (t j b) -> p t j b",
                                              j=KH, b=BL)[:, s0:s1, j, :]
                    srcp = psA[jj][:].rearrange("p (t b) -> p t b", b=BL)
                    nc.scalar.activation(dest, srcp, ACT.Copy,
                                         bias=-SCALE * LEAK, scale=SCALE)

        # ================= Phase B: sequential scan =================
        # psB is [128, 64]: cols 0-31 accumulate the k=0,1 contributions
        # (complete at 50% of the burst), cols 32-63 the k=2,3 ones.
        # Burst order kg-outer / j / kk so each 2-matmul accumulation
        # group is consecutive, and the next step's kg=0 matmuls only
        # need ge01 of the fresh spikes.
        nc.vector.tensor_add(v1[:], v1[:], xproj[:, 0:32])
        psB_prev = None
        for t in range(t_steps):
            sl = slice(32 * t, 32 * t + 32)
            sl0 = slice(32 * t, 32 * t + 16)
            sl1 = slice(32 * t + 16, 32 * t + 32)
            if psB_prev is not None:
                nc.vector.tensor_add(v1[:], v1[:], psB_prev[:, 0:32])
                nc.vector.tensor_add(v1[:], v1[:], psB_prev[:, 32:64])
            nc.vector.tensor_scalar(spk_store[:, sl0], v1[:, 0:16], TH_S, None,
                                    alu.is_ge)
            nc.vector.tensor_scalar(spk_store[:, sl1], v1[:, 16:32], TH_S, None,
                                    alu.is_ge)
            # subtractive reset (DVE), then next xp pre-add on Pool
            nc.vector.scalar_tensor_tensor(v1[:], spk_store[:, sl], -TH_S, v1[:],
                                           alu.mult, alu.add)
            if t + 1 < t_steps:
                nc.gpsimd.tensor_add(v1[:], v1[:],
                                     xproj[:, 32 * (t + 1):32 * (t + 1) + 32])

            # ---- PE burst: rec for step t ----
            psB = psB_pool.tile([P, 2 * KH * BL], f32, tag="psB", name="psB")
            for kg in range(2):
                for j in range(KH):
                    for kk in range(2):
                        k = 2 * kg + kk
                        nc.tensor.matmul(
                            psB[:, 32 * kg + BL * j:32 * kg + BL * (j + 1)],
                            wr_sb[k][:][:, j * P:(j + 1) * P],
                            spk_store[:, 32 * t + BL * k:32 * t + BL * (k + 1)],
                            start=(kk == 0), stop=(kk == 1))
            psB_prev = psB

        # ================= Phase C: v2 accumulator =================
        with tc.tile_pool(name="psV", bufs=1, space="PSUM") as psV_pool, \
             tc.tile_pool(name="phC", bufs=1) as phC_pool:
            d_all = phC_pool.tile([O, t_steps * BL], f32, tag="d_all",
                                  name="d_all")
            nsteps = min(NTILE // BL, t_steps)
            nC = (t_steps + nsteps - 1) // nsteps
            spk_r = spk_store[:].rearrange("p (t c b) -> p t c b", c=KH, b=BL)
            for n in range(nC):
                c0, c1 = n * nsteps, min((n + 1) * nsteps, t_steps)
                psV = psV_pool.tile([O, (c1 - c0) * BL], f32, tag="psV",
                                    name="psV", padded_shape=[O, NTILE])
                for k in range(KH):
                    rhs = spk_r[:, c0:c1, k, :]
                    nc.tensor.matmul(psV[:], w2_sb[k][:], rhs,
                                     start=(k == 0), stop=(k == KH - 1))
                # d = (i2 - OL) / T   (scan and sum are homogeneous in scale)
                nc.vector.tensor_scalar(
                    d_all[:, c0 * BL:c1 * BL], psV[:],
                    -OUTPUT_LEAK, 1.0 / float(T), alu.add, alu.mult)
            # rearrange [O, (t b)] -> [B*O, t] via per-b SBUF->SBUF DMA
            st2 = phC_pool.tile([O * BL, t_steps], f32, tag="st2", name="st2")
            zeros = phC_pool.tile([O * BL, t_steps], f32, tag="zeros",
                                  name="zeros")
            v2a = phC_pool.tile([O * BL, t_steps], f32, tag="v2a", name="v2a")
            osum88 = phC_pool.tile([O * BL, 1], f32, tag="osum88", name="osum88")
            nc.vector.memset(zeros[:], 0.0)
            d_r = d_all[:].rearrange("o (t b) -> o b t", b=BL)
            st2_r = st2[:].rearrange("(b o) t -> b o t", o=O)
            for b in range(BL):
                nc.sync.dma_start(out=st2_r[b, :, :], in_=d_r[:, b, :])
            # v2_t = max(v2_{t-1} + d_t, 0): one scan along t per (o,b) row
            nc.vector.tensor_tensor_scan(v2a[:], st2[:], zeros[:], 0.0,
                                         alu.add, alu.max)
            nc.vector.tensor_reduce(out=osum88[:], in_=v2a[:],
                                    axis=mybir.AxisListType.X, op=alu.add)
            nc.sync.dma_start(out=out_d.ap()[:, :].rearrange("o b -> b o"),
                              in_=osum88[:])

        if debug_out:
            nc.sync.dma_start(out=v1_d.ap()[:, :], in_=v1[:])
            nc.sync.dma_start(out=xp_d.ap()[:, :], in_=xproj[:])

    nc.compile()
    return nc


def prep_inputs(x, w1, w_rec, w2, mode=MODE):
    """Build per-core input maps. Host-side transposes/splits (not timed)."""
    x = np.ascontiguousarray(x, dtype=np.float32)
    w1 = np.ascontiguousarray(w1, dtype=np.float32)
    w_rec = np.ascontiguousarray(w_rec, dtype=np.float32)
    w2 = np.ascontiguousarray(w2, dtype=np.float32)

    if mode == "fp32":
        wrt = np.ascontiguousarray(w_rec.T) * np.float32(SCALE)
        w2t = np.ascontiguousarray(w2.T)
    else:
        wrt = (w_rec.T * SCALE).astype(np.float16)
        w2t = w2.T.astype(np.float16)

    in_maps = []
    if mode == "fp16":
        w1h = w1.T.astype(np.float16)                              # [C, H]
        w1l = ((w1.T - w1h.astype(np.float32)) * LO_SHIFT).astype(np.float16)
        for c in range(NCORES):
            xc = x[c * BL:(c + 1) * BL]                            # [BL, T, C]
            xt = np.ascontiguousarray(xc.transpose(2, 1, 0).reshape(C, BT))
            xh = xt.astype(np.float16)
            xl = (xt - xh.astype(np.float32)).astype(np.float16)
            xs = (xh.astype(np.float32) / LO_SHIFT).astype(np.float16)
            in_maps.append({"xh": xh, "xl": xl, "xs": xs,
                            "w1h": w1h, "w1l": w1l, "wrt": wrt, "w2t": w2t})
    else:
        w1t = np.ascontiguousarray(w1.T)
        for c in range(NCORES):
            xc = x[c * BL:(c + 1) * BL]
            xt = np.ascontiguousarray(xc.transpose(2, 1, 0).reshape(C, BT))
            in_maps.append({"xt": xt, "w1t": w1t, "wrt": wrt, "w2t": w2t})
    return in_maps


_LAST = {"exec_time_ns": None, "results": None}


def kernel(x, w1, w_rec, w2):
    from concourse.bass_utils import run_bass_kernel_spmd

    nc = build_nc()
    in_maps = prep_inputs(x, w1, w_rec, w2)
    trace = os.environ.get("SNN_TRACE", "0") == "1"
    if trace:
        try:
            import antenv
            if "/opt/trn_rl_repo/antenv" not in antenv.__path__:
                antenv.__path__.append("/opt/trn_rl_repo/antenv")
            import antenv.axon_hooks  # noqa: F401
        except Exception:
            trace = False
    res = run_bass_kernel_spmd(nc, in_maps, list(range(NCORES)), trace=trace)
    _LAST["exec_time_ns"] = res.exec_time_ns
    _LAST["results"] = res
    out = np.empty((B, O), dtype=np.float32)
    for c in range(NCORES):
        out[c * BL:(c + 1) * BL, :] = res.results[c]["out"].T
    return out


# revision 21
# speedup vs baseline: 1.0262x; 1.0262x over previous
"""Trainium2 Bass kernel for the DVS-SNN problem.

Model (per reference):
    for t in 0..T-1:
        i1 = x_t @ w1.T + spk @ w_rec.T
        v1 = v1 + i1 - LEAK ; spk = (v1 >= 1) ; v1 -= spk
        v2 = max(v2 + spk @ w2.T - OUTPUT_LEAK, 0) ; out_sum += v2
    return out_sum / T

Strategy: data-parallel over batch (64 = 8 cores x 8). Per core:
  Phase A (time-parallel): XprojT[h, t*8+b] = scale*((x_t @ w1.T).T - LEAK)
      with x pre-transposed on host to xt[c, t*8+b]; fp16 hi/lo 3-pass
      (exact to ~fp32). Runs in j-pair passes, double-buffered PSUM, and
      interleaves into phase B's PE gaps (Tile scheduler).
  Phase B (sequential scan over T): state transposed [H-chunkwise on
      partitions, B on free]: tiles [128, 32], free = 8*h_chunk + b.
      rec.T = w_rec @ spk.T via 16 matmuls (stationary = w_rec.T chunks,
      fp16 scaled by 64 to stay out of subnormals; v1 dynamics scaled by
      64, numerically transparent). Burst is ordered k-outer so the next
      step's first matmuls only need the first half of the new spikes.
      Spikes are written straight into spk_store (fp16) which feeds both
      the matmuls and phase C.
  Phase C (time-parallel): i2 = spk @ w2.T batched; the v2 relu
      accumulator is a first-order recurrence = one tensor_tensor_scan
      (state = max(state + d_t, 0)) over [B*O, T] layout; reduce-sum
      gives out_sum.

Modes (env SNN_MODE): "fp32" exact | "fp16" (default, ~1.7e-3).
"""

import os
import numpy as np

B, T, C, H, O = 64, 256, 2048, 512, 11
THRESHOLD = 1.0
LEAK = 0.003
OUTPUT_LEAK = LEAK * 0.5

NCORES = 8
BL = B // NCORES          # batch per core = 8
BT = T * BL               # 2048 moving columns per core
P = 128
KC = C // P               # 16 contraction chunks for phase A
KH = H // P               # 4 H chunks
NTILE = 512               # phase A moving tile (64 timesteps x 8 batch)

MODE = os.environ.get("SNN_MODE", "fp16")

# fp16 mode: scale v1 dynamics so fp16(SCALE*w_rec) avoids subnormals.
SCALE = 1.0 if MODE == "fp32" else 64.0
LO_SHIFT = 2048.0  # 2^11: scaling for the w1 low-part in fp16 3-pass


def build_nc(t_steps=T, mode=MODE, debug_out=False):
    """Build the Bass program (same program for all 8 cores)."""
    from contextlib import ExitStack

    import concourse.tile as tile
    from concourse import bacc, mybir

    f32 = mybir.dt.float32
    f16 = mybir.dt.float16
    wdt = f32 if mode == "fp32" else f16
    alu = mybir.AluOpType
    ACT = mybir.ActivationFunctionType

    nc = bacc.Bacc("TRN2", target_bir_lowering=False, debug=False,
                   num_devices=NCORES)

    # ---- DRAM I/O ----
    if mode == "fp16":
        xh_d = nc.dram_tensor("xh", [C, BT], f16, kind="ExternalInput")
        xl_d = nc.dram_tensor("xl", [C, BT], f16, kind="ExternalInput")
        xs_d = nc.dram_tensor("xs", [C, BT], f16, kind="ExternalInput")
        w1h_d = nc.dram_tensor("w1h", [C, H], f16, kind="ExternalInput")
        w1l_d = nc.dram_tensor("w1l", [C, H], f16, kind="ExternalInput")
    else:
        adt = f32
        xt_d = nc.dram_tensor("xt", [C, BT], adt, kind="ExternalInput")
        w1t_d = nc.dram_tensor("w1t", [C, H], adt, kind="ExternalInput")
    wrt_d = nc.dram_tensor("wrt", [H, H], wdt, kind="ExternalInput")
    w2t_d = nc.dram_tensor("w2t", [H, O], wdt, kind="ExternalInput")
    out_d = nc.dram_tensor("out", [O, BL], f32, kind="ExternalOutput")
    if debug_out:
        v1_d = nc.dram_tensor("v1_dbg", [P, KH * BL], f32, kind="ExternalOutput")
        xp_d = nc.dram_tensor("xp_dbg", [P, 32 * t_steps], f32, kind="ExternalOutput")

    TH_S = SCALE * THRESHOLD
    spk_dt = wdt

    with tile.TileContext(nc) as tc, ExitStack() as ctx:
        # ---- persistent tiles (one pool, one slot per tag) ----
        perm = ctx.enter_context(tc.tile_pool(name="perm", bufs=1))

        def ptile(shape, dt_, tag):
            return perm.tile(shape, dt_, tag=tag, name=tag)

        xproj = ptile([P, 32 * t_steps], f32, "xproj")
        spk_store = ptile([P, 32 * t_steps], spk_dt, "spk_store")
        v1 = ptile([P, KH * BL], f32, "v1")

        w1_sb = []
        if mode == "fp16":
            for k in range(KC):
                w1h_k = ptile([P, H], f16, f"w1h_{k}")
                w1l_k = ptile([P, H], f16, f"w1l_{k}")
                nc.sync.dma_start(out=w1h_k[:], in_=w1h_d.ap()[k * P:(k + 1) * P, :])
                nc.sync.dma_start(out=w1l_k[:], in_=w1l_d.ap()[k * P:(k + 1) * P, :])
                w1_sb.append((w1h_k, w1l_k))
        else:
            for k in range(KC):
                w1_k = ptile([P, H], adt, f"w1_{k}")
                nc.sync.dma_start(out=w1_k[:], in_=w1t_d.ap()[k * P:(k + 1) * P, :])
                w1_sb.append(w1_k)

        wr_sb = []
        w2_sb = []
        for k in range(KH):
            wr_k = ptile([P, H], wdt, f"wr_{k}")
            nc.sync.dma_start(out=wr_k[:], in_=wrt_d.ap()[k * P:(k + 1) * P, :])
            wr_sb.append(wr_k)
            w2_k = ptile([P, O], wdt, f"w2_{k}")
            nc.sync.dma_start(out=w2_k[:], in_=w2t_d.ap()[k * P:(k + 1) * P, :])
            w2_sb.append(w2_k)

        nc.vector.memset(v1[:], 0.0)

        from contextlib import ExitStack as _ES
        xt_pool = ctx.enter_context(tc.tile_pool(name="xt", bufs=4))
        psB_pool = ctx.enter_context(tc.tile_pool(name="psB", bufs=2, space="PSUM"))
        psA_ctx = _ES()
        psA_pool = psA_ctx.enter_context(
            tc.tile_pool(name="psA", bufs=2, space="PSUM"))

        # ================= Phase A: XprojT =================
        # xproj[p, 32t + 8j + b] = SCALE * ((x_t @ w1.T)[b, 128j+p] - LEAK)
        # j-pair passes: each pass computes 2 of the 4 H-chunks for a
        # column tile (2 PSUM tags x bufs=2 = 4 banks, double-buffered).
        # x tiles are re-streamed once per pass (2x DMA, still hidden).
        # A small leading tile (8 steps) lets phase B start early.
        if t_steps <= NTILE // BL:
            sched = [(0, t_steps)]
        else:
            sched = [(0, 8)]
            s = 8
            while s < t_steps:
                sched.append((s, min(s + NTILE // BL, t_steps)))
                s = min(s + NTILE // BL, t_steps)
        for (s0, s1) in sched:
            ntile = (s1 - s0) * BL
            nsl = slice(s0 * BL, s1 * BL)
            for jp in range(2):
                psA = [psA_pool.tile([P, ntile], f32, tag=f"psA{jj}",
                                     name=f"psA{jj}", padded_shape=[P, NTILE])
                       for jj in range(2)]
                for k in range(KC):
                    csl = slice(k * P, (k + 1) * P)
                    if mode == "fp16":
                        xh_t = xt_pool.tile([P, ntile], f16, tag="xh",
                                            name="xh_t", padded_shape=[P, NTILE])
                        xl_t = xt_pool.tile([P, ntile], f16, tag="xl",
                                            name="xl_t", padded_shape=[P, NTILE])
                        xs_t = xt_pool.tile([P, ntile], f16, tag="xs",
                                            name="xs_t", padded_shape=[P, NTILE])
                        nc.sync.dma_start(out=xh_t[:], in_=xh_d.ap()[csl, nsl])
                        nc.sync.dma_start(out=xl_t[:], in_=xl_d.ap()[csl, nsl])
                        nc.sync.dma_start(out=xs_t[:], in_=xs_d.ap()[csl, nsl])
                        for jj in range(2):
                            j = 2 * jp + jj
                            st_h = w1_sb[k][0][:][:, j * P:(j + 1) * P]
                            st_l = w1_sb[k][1][:][:, j * P:(j + 1) * P]
                            nc.tensor.matmul(psA[jj][:], st_h, xh_t[:],
                                             start=(k == 0), stop=False)
                            nc.tensor.matmul(psA[jj][:], st_h, xl_t[:],
                                             start=False, stop=False)
                            nc.tensor.matmul(psA[jj][:], st_l, xs_t[:],
                                             start=False, stop=(k == KC - 1))
                    else:
                        xt_t = xt_pool.tile([P, ntile], adt, tag="xt",
                                            name="xt_t", padded_shape=[P, NTILE])
                        nc.sync.dma_start(out=xt_t[:], in_=xt_d.ap()[csl, nsl])
                        for jj in range(2):
                            j = 2 * jp + jj
                            st = w1_sb[k][:][:, j * P:(j + 1) * P]
                            nc.tensor.matmul(psA[jj][:], st, xt_t[:],
                                             start=(k == 0), stop=(k == KC - 1))
                # copy PSUM -> xproj (strided dest), fusing scale and leak
                for jj in range(2):
                    j = 2 * jp + jj
                    dest = xproj[:].rearrange("p (t j b) -> p t j b",
                                              j=KH, b=BL)[:, s0:s1, j, :]
                    srcp = psA[jj][:].rearrange("p (t b) -> p t b", b=BL)
                    nc.scalar.activation(dest, srcp, ACT.Copy,
                                         bias=-SCALE * LEAK, scale=SCALE)

        # ================= Phase B: sequential scan =================
        # psB is [128, 64]: cols 0-31 accumulate the k=0,1 contributions
        # (complete at 50% of the burst), cols 32-63 the k=2,3 ones.
        # Burst order kg-outer / j / kk so each 2-matmul accumulation
        # group is consecutive, and the next step's kg=0 matmuls only
        # need ge01 of the fresh spikes.
        nc.vector.tensor_add(v1[:], v1[:], xproj[:, 0:32])
        psB_prev = None
        for t in range(t_steps):
            sl = slice(32 * t, 32 * t + 32)
            sl0 = slice(32 * t, 32 * t + 16)
            sl1 = slice(32 * t + 16, 32 * t + 32)
            if psB_prev is not None:
                nc.vector.tensor_add(v1[:], v1[:], psB_prev[0][:])
                nc.vector.tensor_add(v1[:], v1[:], psB_prev[1][:])
            nc.vector.tensor_scalar(spk_store[:, sl0], v1[:, 0:16], TH_S, None,
                                    alu.is_ge)
            nc.vector.tensor_scalar(spk_store[:, sl1], v1[:, 16:32], TH_S, None,
                                    alu.is_ge)
            # subtractive reset (DVE), then next xp pre-add on Pool
            nc.vector.scalar_tensor_tensor(v1[:], spk_store[:, sl], -TH_S, v1[:],
                                           alu.mult, alu.add)
            if t + 1 < t_steps:
                nc.gpsimd.tensor_add(v1[:], v1[:],
                                     xproj[:, 32 * (t + 1):32 * (t + 1) + 32])

            # ---- PE burst: rec for step t (two psum tiles, kg halves) ----
            psB = [psB_pool.tile([P, KH * BL], f32, tag=f"psB{kg}",
                                 name=f"psB{kg}") for kg in range(2)]
            for kg in range(2):
                for j in range(KH):
                    for kk in range(2):
                        k = 2 * kg + kk
                        nc.tensor.matmul(
                            psB[kg][:, BL * j:BL * (j + 1)],
                            wr_sb[k][:][:, j * P:(j + 1) * P],
                            spk_store[:, 32 * t + BL * k:32 * t + BL * (k + 1)],
                            start=(kk == 0), stop=(kk == 1))
            psB_prev = psB

        psA_ctx.close()

        # ================= Phase C: v2 accumulator =================
        with tc.tile_pool(name="psV", bufs=1, space="PSUM") as psV_pool, \
             tc.tile_pool(name="phC", bufs=1) as phC_pool:
            d_all = phC_pool.tile([O, t_steps * BL], f32, tag="d_all",
                                  name="d_all")
            nsteps = min(NTILE // BL, t_steps)
            nC = (t_steps + nsteps - 1) // nsteps
            spk_r = spk_store[:].rearrange("p (t c b) -> p t c b", c=KH, b=BL)
            for n in range(nC):
                c0, c1 = n * nsteps, min((n + 1) * nsteps, t_steps)
                psV = psV_pool.tile([O, (c1 - c0) * BL], f32, tag="psV",
                                    name="psV", padded_shape=[O, NTILE])
                for k in range(KH):
                    rhs = spk_r[:, c0:c1, k, :]
                    nc.tensor.matmul(psV[:], w2_sb[k][:], rhs,
                                     start=(k == 0), stop=(k == KH - 1))
                # d = (i2 - OL) / T   (scan and sum are homogeneous in scale)
                nc.vector.tensor_scalar(
                    d_all[:, c0 * BL:c1 * BL], psV[:],
                    -OUTPUT_LEAK, 1.0 / float(T), alu.add, alu.mult)
            # rearrange [O, (t b)] -> [B*O, t] via per-b SBUF->SBUF DMA
            st2 = phC_pool.tile([O * BL, t_steps], f32, tag="st2", name="st2")
            zeros = phC_pool.tile([O * BL, t_steps], f32, tag="zeros",
                                  name="zeros")
            v2a = phC_pool.tile([O * BL, t_steps], f32, tag="v2a", name="v2a")
            osum88 = phC_pool.tile([O * BL, 1], f32, tag="osum88", name="osum88")
            nc.vector.memset(zeros[:], 0.0)
            d_r = d_all[:].rearrange("o (t b) -> o b t", b=BL)
            st2_r = st2[:].rearrange("(b o) t -> b o t", o=O)
            for b in range(BL):
                nc.sync.dma_start(out=st2_r[b, :, :], in_=d_r[:, b, :])
            # v2_t = max(v2_{t-1} + d_t, 0): one scan along t per (o,b) row
            nc.vector.tensor_tensor_scan(v2a[:], st2[:], zeros[:], 0.0,
                                         alu.add, alu.max)
            nc.vector.tensor_reduce(out=osum88[:], in_=v2a[:],
                                    axis=mybir.AxisListType.X, op=alu.add)
            nc.sync.dma_start(out=out_d.ap()[:, :].rearrange("o b -> b o"),
                              in_=osum88[:])

        if debug_out:
            nc.sync.dma_start(out=v1_d.ap()[:, :], in_=v1[:])
            nc.sync.dma_start(out=xp_d.ap()[:, :], in_=xproj[:])

    nc.compile()
    return nc


def prep_inputs(x, w1, w_rec, w2, mode=MODE):
    """Build per-core input maps. Host-side transposes/splits (not timed)."""
    x = np.ascontiguousarray(x, dtype=np.float32)
    w1 = np.ascontiguousarray(w1, dtype=np.float32)
    w_rec = np.ascontiguousarray(w_rec, dtype=np.float32)
    w2 = np.ascontiguousarray(w2, dtype=np.float32)

    if mode == "fp32":
        wrt = np.ascontiguousarray(w_rec.T) * np.float32(SCALE)
        w2t = np.ascontiguousarray(w2.T)
    else:
        wrt = (w_rec.T * SCALE).astype(np.float16)
        w2t = w2.T.astype(np.float16)

    in_maps = []
    if mode == "fp16":
        w1h = w1.T.astype(np.float16)                              # [C, H]
        w1l = ((w1.T - w1h.astype(np.float32)) * LO_SHIFT).astype(np.float16)
        for c in range(NCORES):
            xc = x[c * BL:(c + 1) * BL]                            # [BL, T, C]
            xt = np.ascontiguousarray(xc.transpose(2, 1, 0).reshape(C, BT))
            xh = xt.astype(np.float16)
            xl = (xt - xh.astype(np.float32)).astype(np.float16)
            xs = (xh.astype(np.float32) / LO_SHIFT).astype(np.float16)
            in_maps.append({"xh": xh, "xl": xl, "xs": xs,
                            "w1h": w1h, "w1l": w1l, "wrt": wrt, "w2t": w2t})
    else:
        w1t = np.ascontiguousarray(w1.T)
        for c in range(NCORES):
            xc = x[c * BL:(c + 1) * BL]
            xt = np.ascontiguousarray(xc.transpose(2, 1, 0).reshape(C, BT))
            in_maps.append({"xt": xt, "w1t": w1t, "wrt": wrt, "w2t": w2t})
    return in_maps


_LAST = {"exec_time_ns": None, "results": None}


def kernel(x, w1, w_rec, w2):
    from concourse.bass_utils import run_bass_kernel_spmd

    nc = build_nc()
    in_maps = prep_inputs(x, w1, w_rec, w2)
    trace = os.environ.get("SNN_TRACE", "0") == "1"
    if trace:
        try:
            import antenv
            if "/opt/trn_rl_repo/antenv" not in antenv.__path__:
                antenv.__path__.append("/opt/trn_rl_repo/antenv")
            import antenv.axon_hooks  # noqa: F401
        except Exception:
            trace = False
    res = run_bass_kernel_spmd(nc, in_maps, list(range(NCORES)), trace=trace)
    _LAST["exec_time_ns"] = res.exec_time_ns
    _LAST["results"] = res
    out = np.empty((B, O), dtype=np.float32)
    for c in range(NCORES):
        out[c * BL:(c + 1) * BL, :] = res.results[c]["out"].T
    return out
